# revision 3
# baseline (speedup 1.0000x reference)
"""NPMLPHead sampling kernel for Trainium2 (Bass/Tile), SPMD over 8 cores.

Strategy (data-parallel over batch, per sharding hint):
  - B=16 images -> 2 per core; full (tiny) MLP weights replicated per core.
  - Each core's kernel gathers its 128 patch vectors per pyramid level
    directly from HBM with strided DMAs (one DMA per patch index: the
    patch vector feat[:, :, q] is a [c-partition x (b,chunk)] strided
    pattern of 4B elements), landing transposed as xT[c, p] in SBUF --
    exactly the layout the TensorEngine needs.
  - MLP: hT[n,p] = relu(w1.T x + b1) via PE matmuls (biases folded in as
    rank-1 K=1 matmuls), y[p,m] = hT.T w2 + b2, then L2-normalize rows
    with ACT/DVE ops, DMA out.
  - No cross-core communication; host concatenates the 8 output shards.
"""

import sys

sys.path.insert(0, "/opt/trn_rl_repo")

import numpy as np

B = 16
N_CORES = 8
B_LOC = B // N_CORES  # 2
P = 128  # NUM_PATCHES
NCD = 256  # MLP width
LEVELS = [(256, 128), (512, 64), (1024, 32)]  # (C, H) per level
EPS = 1e-7


def _build(idx_vals):
    """Build the per-core Bass kernel. idx_vals: 3 int arrays of 128 patch ids."""
    import concourse.bass as bass
    import concourse.tile as tile
    from concourse import bacc, mybir

    f32 = mybir.dt.float32
    AF = mybir.ActivationFunctionType

    nc = bacc.Bacc(None)

    feats, w1s, b1s, w2s, b2s = [], [], [], [], []
    for l, (C, H) in enumerate(LEVELS):
        feats.append(
            nc.dram_tensor(f"feat{l}", [B_LOC, C, H, H], f32, kind="ExternalInput")
        )
        w1s.append(nc.dram_tensor(f"w1_{l}", [C, NCD], f32, kind="ExternalInput"))
        b1s.append(nc.dram_tensor(f"b1_{l}", [NCD], f32, kind="ExternalInput"))
        w2s.append(nc.dram_tensor(f"w2_{l}", [NCD, NCD], f32, kind="ExternalInput"))
        b2s.append(nc.dram_tensor(f"b2_{l}", [NCD], f32, kind="ExternalInput"))
    out = nc.dram_tensor("out", [3, B_LOC, P, NCD], f32, kind="ExternalOutput")

    with tile.TileContext(nc) as tc:
        with (
            tc.tile_pool(name="consts", bufs=1) as consts,
            tc.tile_pool(name="xt", bufs=1) as xtp,
            tc.tile_pool(name="work", bufs=4) as work,
            tc.tile_pool(name="psum", bufs=4, space=bass.MemorySpace.PSUM) as psum,
        ):
            ones = consts.tile([1, P], f32, tag="ones")
            nc.vector.memset(ones[:], 1.0)

            w1_sb, w2_sb, b1_sb, b2_sb, xts = [], [], [], [], []
            for l, (C, H) in enumerate(LEVELS):
                n_ch = C // 128
                t = consts.tile([128, n_ch * NCD], f32, tag=f"w1_{l}", name=f"w1sb{l}")
                nc.sync.dma_start(
                    t[:].rearrange("cp (ch n) -> cp ch n", n=NCD),
                    w1s[l][:].rearrange("(ch cp) n -> cp ch n", cp=128),
                )
                w1_sb.append(t)
                t = consts.tile([128, 2 * NCD], f32, tag=f"w2_{l}", name=f"w2sb{l}")
                nc.sync.dma_start(
                    t[:].rearrange("cp (ch n) -> cp ch n", n=NCD),
                    w2s[l][:].rearrange("(ch cp) n -> cp ch n", cp=128),
                )
                w2_sb.append(t)
                t = consts.tile([1, NCD], f32, tag=f"b1_{l}", name=f"b1sb{l}")
                nc.sync.dma_start(t[:], b1s[l][:].rearrange("(o n) -> o n", o=1))
                b1_sb.append(t)
                t = consts.tile([1, NCD], f32, tag=f"b2_{l}", name=f"b2sb{l}")
                nc.sync.dma_start(t[:], b2s[l][:].rearrange("(o n) -> o n", o=1))
                b2_sb.append(t)
                # Gathered patches, transposed: [c_part, (b, chunk) x patch]
                xts.append(xtp.tile([128, B_LOC * n_ch * P], f32, tag=f"xt_{l}", name=f"xt{l}"))

            # --- sparse gather: one DMA per (level, patch) ---
            gather_eng = [nc.gpsimd, nc.sync, nc.scalar]  # L0, L1, L2
            with nc.allow_non_contiguous_dma("sparse patch gather"):
                for l, (C, H) in enumerate(LEVELS):
                    n_ch = C // 128
                    # src: [cp, (b ch), q]  steps: (HW, 128*HW, 1)
                    src3 = feats[l][:].rearrange(
                        "b (ch cp) h w -> cp (b ch) (h w)", cp=128
                    )
                    dst3 = xts[l][:].rearrange("c (bc pp) -> c bc pp", pp=P)
                    eng = gather_eng[l]
                    for i, q in enumerate(idx_vals[l]):
                        eng.dma_start(dst3[:, :, i], src3[:, :, int(q)])

            # --- MLP + l2norm per (level, image) ---
            for l, (C, H) in enumerate(LEVELS):
                n_ch = C // 128
                for b in range(B_LOC):
                    hts = []
                    for half in range(2):
                        ph = psum.tile([128, P], f32, tag="ph")
                        for ch in range(n_ch):
                            o = ch * NCD + half * 128
                            nc.tensor.matmul(
                                ph[:],
                                w1_sb[l][:, o : o + 128],
                                xts[l][:, (b * n_ch + ch) * P : (b * n_ch + ch + 1) * P],
                                start=(ch == 0),
                                stop=False,
                            )
                        # + b1 (rank-1: b1_half^T . ones)
                        nc.tensor.matmul(
                            ph[:],
                            b1_sb[l][0:1, half * 128 : half * 128 + 128],
                            ones[0:1, :],
                            start=False,
                            stop=True,
                        )
                        ht = work.tile([128, P], f32, tag="ht")
                        nc.scalar.activation(ht[:], ph[:], AF.Relu)
                        hts.append(ht)

                    py = psum.tile([128, NCD], f32, tag="py")
                    for half in range(2):
                        nc.tensor.matmul(
                            py[:],
                            hts[half][:],
                            w2_sb[l][:, half * NCD : (half + 1) * NCD],
                            start=(half == 0),
                            stop=False,
                        )
                    # + b2 (rank-1: ones^T . b2)
                    nc.tensor.matmul(
                        py[:], ones[0:1, :], b2_sb[l][0:1, :], start=False, stop=True
                    )

                    sq = work.tile([128, NCD], f32, tag="sq")
                    ssq = work.tile([128, 1], f32, tag="ssq")
                    nc.scalar.activation(sq[:], py[:], AF.Square, accum_out=ssq[:])
                    nrm = work.tile([128, 1], f32, tag="nrm")
                    nc.scalar.sqrt(nrm[:], ssq[:])
                    nrm2 = work.tile([128, 1], f32, tag="nrm2")
                    nc.vector.tensor_scalar_add(nrm2[:], nrm[:], EPS)
                    inv = work.tile([128, 1], f32, tag="inv")
                    nc.vector.reciprocal(inv[:], nrm2[:])
                    yo = work.tile([128, NCD], f32, tag="yo")
                    nc.scalar.mul(yo[:], py[:], inv[:])
                    nc.sync.dma_start(out[l, b], yo[:])

    nc.compile()
    return nc


def _run(inputs, trace=False):
    from concourse.bass_utils import run_bass_kernel_spmd

    feats = [np.ascontiguousarray(np.asarray(inputs[f"feat{l}"])) for l in range(3)]
    idxs = [np.asarray(inputs[f"idx{l}"]).astype(np.int64) for l in range(3)]
    nc = _build(idxs)

    in_maps = []
    for c in range(N_CORES):
        m = {}
        for l in range(3):
            m[f"feat{l}"] = feats[l][c * B_LOC : (c + 1) * B_LOC]
            m[f"w1_{l}"] = np.asarray(inputs[f"w1_{l}"])
            m[f"b1_{l}"] = np.asarray(inputs[f"b1_{l}"])
            m[f"w2_{l}"] = np.asarray(inputs[f"w2_{l}"])
            m[f"b2_{l}"] = np.asarray(inputs[f"b2_{l}"])
        in_maps.append(m)

    res = run_bass_kernel_spmd(
        nc, in_maps, core_ids=list(range(N_CORES)), trace=trace
    )
    full = np.concatenate([r["out"] for r in res.results], axis=1)
    return full.astype(np.float32), res


def kernel(**inputs) -> np.ndarray:
    out, _ = _run(inputs, trace=False)
    return out


# revision 10
# speedup vs baseline: 1.6047x; 1.6047x over previous
"""NPMLPHead sampling kernel for Trainium2 (Bass/Tile), SPMD over 8 cores.

Strategy (data-parallel over batch, per sharding hint):
  - B=16 images -> 2 per core; full (tiny) MLP weights replicated per core.
  - Levels 0/1 (sparse: 128 of 16K/4K positions): gather the patch vectors
    straight from HBM with strided DMAs (one DMA per patch; elements land
    transposed as xT[c, p] in SBUF). L0 rides HWDGE; L1 mostly SWDGE whose
    descriptors coalesce 16x, with a tail on the second HWDGE ring to
    balance the Pool sequencer against the DMA engines.
  - Level 2 (dense-ish: 128 of 1024): stream the whole shard and use
    associativity  y = S^T ((relu(W1^T T))^T W2)  -- layer-1 and layer-2
    computed over ALL positions keep the contraction dim on partitions in
    the native [C, HW] layout (no transposes), and the one-hot select S
    becomes a single PE matmul once q lands on partitions.
  - All matmuls in float32r (single-pass fp32, 4x faster; ~tf32 rounding).
  - L2-normalize rows with ACT/DVE ops, DMA out; host concatenates shards.
"""

import sys

sys.path.insert(0, "/opt/trn_rl_repo")

import numpy as np

B = 16
N_CORES = 8
B_LOC = B // N_CORES  # 2
P = 128  # NUM_PATCHES
NCD = 256  # MLP width
LEVELS = [(256, 128), (512, 64), (1024, 32)]  # (C, H) per level
EPS = 1e-7
L1_POOL_COUNT = 100  # L1 gather DMAs on SWDGE; the rest on HWDGE (balance)


def _build(idx_vals):
    """Build the per-core Bass kernel. idx_vals: 3 int arrays of 128 patch ids."""
    import concourse.bass as bass
    import concourse.tile as tile
    from concourse import bacc, mybir

    f32 = mybir.dt.float32
    fr = mybir.dt.float32r
    AF = mybir.ActivationFunctionType

    nc = bacc.Bacc(None)

    feats, w1s, b1s, w2s, b2s = [], [], [], [], []
    for l, (C, H) in enumerate(LEVELS):
        feats.append(
            nc.dram_tensor(f"feat{l}", [B_LOC, C, H, H], fr, kind="ExternalInput")
        )
        w1s.append(nc.dram_tensor(f"w1_{l}", [C, NCD], fr, kind="ExternalInput"))
        b1s.append(nc.dram_tensor(f"b1_{l}", [NCD], fr, kind="ExternalInput"))
        w2s.append(nc.dram_tensor(f"w2_{l}", [NCD, NCD], fr, kind="ExternalInput"))
        b2s.append(nc.dram_tensor(f"b2_{l}", [NCD], fr, kind="ExternalInput"))
    # one-hot select for level 2: oh[qc, ql, p] = (idx2[p] == qc*128+ql)
    oh = nc.dram_tensor("oh2", [8, 128, P], fr, kind="ExternalInput")
    out = nc.dram_tensor("out", [3, B_LOC, P, NCD], f32, kind="ExternalOutput")

    C2, H2 = LEVELS[2]
    HW2 = H2 * H2  # 1024
    NCH2 = C2 // 128  # 8
    QC2 = HW2 // 128  # 8 q-chunks

    with tile.TileContext(nc) as tc:
        with (
            tc.tile_pool(name="consts", bufs=1) as consts,
            tc.tile_pool(name="xt", bufs=1) as xtp,
            tc.tile_pool(name="work", bufs=4) as work,
            tc.tile_pool(name="psum", bufs=2, space=bass.MemorySpace.PSUM) as psum,
        ):
            ones_f = consts.tile([1, 512], f32, tag="ones_f")
            nc.vector.memset(ones_f[:], 1.0)
            ones = consts.tile([1, 512], fr, tag="ones")
            nc.scalar.copy(ones[:], ones_f[:])

            w1_sb, w2_sb, b1_sb, b2_sb, xts = [], [], [], [], []
            for l, (C, H) in enumerate(LEVELS):
                n_ch = C // 128
                t = consts.tile([128, n_ch * NCD], fr, tag=f"w1_{l}", name=f"w1sb{l}")
                nc.sync.dma_start(
                    t[:].rearrange("cp (ch n) -> cp ch n", n=NCD),
                    w1s[l][:].rearrange("(ch cp) n -> cp ch n", cp=128),
                )
                w1_sb.append(t)
                t = consts.tile([128, 2 * NCD], fr, tag=f"w2_{l}", name=f"w2sb{l}")
                nc.sync.dma_start(
                    t[:].rearrange("cp (ch n) -> cp ch n", n=NCD),
                    w2s[l][:].rearrange("(ch cp) n -> cp ch n", cp=128),
                )
                w2_sb.append(t)
                t = consts.tile([1, NCD], fr, tag=f"b1_{l}", name=f"b1sb{l}")
                nc.sync.dma_start(t[:], b1s[l][:].rearrange("(o n) -> o n", o=1))
                b1_sb.append(t)
                t = consts.tile([1, NCD], fr, tag=f"b2_{l}", name=f"b2sb{l}")
                nc.sync.dma_start(t[:], b2s[l][:].rearrange("(o n) -> o n", o=1))
                b2_sb.append(t)
                if l < 2:
                    # gathered patches, transposed: [c_part, (b, chunk) x patch]
                    xts.append(
                        xtp.tile(
                            [128, B_LOC * n_ch * P], fr, tag=f"xt_{l}", name=f"xt{l}"
                        )
                    )

            oh_sb = consts.tile([128, QC2 * P], fr, tag="oh2")
            nc.sync.dma_start(
                oh_sb[:].rearrange("ql (qc p) -> ql qc p", p=P),
                oh[:].rearrange("qc ql p -> ql qc p"),
            )

            # --- L0/L1 sparse gather: one DMA per (level, patch) ---
            with nc.allow_non_contiguous_dma("sparse patch gather"):
                for l in (0, 1):
                    C, H = LEVELS[l]
                    n_ch = C // 128
                    src3 = feats[l][:].rearrange(
                        "b (ch cp) h w -> cp (b ch) (h w)", cp=128
                    )
                    dst3 = xts[l][:].rearrange("c (bc pp) -> c bc pp", pp=P)
                    for i, q in enumerate(idx_vals[l]):
                        if l == 0:
                            eng = nc.sync
                        else:
                            eng = nc.gpsimd if i < L1_POOL_COUNT else nc.scalar
                        eng.dma_start(dst3[:, :, i], src3[:, :, int(q)])

            # --- L2: stream whole shard (one DMA per image) ---
            t2s = []
            for b in range(B_LOC):
                t2 = xtp.tile([128, NCH2 * HW2], fr, tag=f"t2_{b}", name=f"t2_{b}")
                nc.scalar.dma_start(
                    t2[:].rearrange("cp (cc hw) -> cp cc hw", hw=HW2),
                    feats[2][b].rearrange("(cc cp) h w -> cp cc (h w)", cp=128),
                )
                t2s.append(t2)

            # --- MLP for L0/L1 (both images batched into N=256) ---
            for l in (0, 1):
                C, H = LEVELS[l]
                n_ch = C // 128
                x4 = xts[l][:].rearrange("c (b ch p) -> c ch b p", b=B_LOC, p=P)
                hts = []
                for half in range(2):
                    ph = psum.tile([128, B_LOC * P], f32, tag="ph", name="ph")
                    for ch in range(n_ch):
                        o = ch * NCD + half * 128
                        nc.tensor.matmul(
                            ph[:],
                            w1_sb[l][:, o : o + 128],
                            x4[:, ch],
                            start=(ch == 0),
                            stop=False,
                        )
                    nc.tensor.matmul(  # + b1 (rank-1)
                        ph[:],
                        b1_sb[l][0:1, half * 128 : half * 128 + 128],
                        ones[0:1, 0 : B_LOC * P],
                        start=False,
                        stop=True,
                    )
                    ht = work.tile([128, B_LOC * P], fr, tag="ht", name="ht")
                    nc.scalar.activation(ht[:], ph[:], AF.Relu)
                    hts.append(ht)

                for b in range(B_LOC):
                    py = psum.tile([128, NCD], f32, tag="py", name="py")
                    for half in range(2):
                        nc.tensor.matmul(
                            py[:],
                            hts[half][:, b * P : (b + 1) * P],
                            w2_sb[l][:, half * NCD : (half + 1) * NCD],
                            start=(half == 0),
                            stop=False,
                        )
                    nc.tensor.matmul(  # + b2 (rank-1)
                        py[:],
                        ones[0:1, 0:P],
                        b2_sb[l][0:1, :],
                        start=False,
                        stop=True,
                    )
                    _norm_and_store(nc, tc, work, AF, f32, py, out, l, b)

            # --- L2 full-compute: G=W1^T T (all q), H=relu(G), K=H^T W2,
            #     y = S^T K (one-hot select once q is on partitions) ---
            for b in range(B_LOC):
                t2 = t2s[b]
                h2 = xtp.tile([128, 2 * HW2], fr, tag=f"h2_{b}", name=f"h2_{b}")
                for half in range(2):
                    for qn in range(2):  # HW2 in two 512-wide slabs
                        g = psum.tile([128, 512], f32, tag="g", name="g")
                        for cc in range(NCH2):
                            nc.tensor.matmul(
                                g[:],
                                w1_sb[2][:, cc * NCD + half * 128 : cc * NCD + half * 128 + 128],
                                t2[:, cc * HW2 + qn * 512 : cc * HW2 + qn * 512 + 512],
                                start=(cc == 0),
                                stop=False,
                            )
                        nc.tensor.matmul(  # + b1 broadcast over all q
                            g[:],
                            b1_sb[2][0:1, half * 128 : half * 128 + 128],
                            ones[0:1, 0:512],
                            start=False,
                            stop=True,
                        )
                        nc.scalar.activation(
                            h2[:, (half * 2 + qn) * 512 : (half * 2 + qn) * 512 + 512],
                            g[:],
                            AF.Relu,
                        )

                py = psum.tile([128, NCD], f32, tag="py", name="py2")  # shares "py" bank slots
                for qc in range(QC2):
                    k = psum.tile([128, NCD], f32, tag="k", name="k")
                    for half in range(2):
                        o = (half * 2 + qc // 4) * 512 + (qc % 4) * 128
                        nc.tensor.matmul(
                            k[:],
                            h2[:, o : o + 128],
                            w2_sb[2][:, half * NCD : (half + 1) * NCD],
                            start=(half == 0),
                            stop=False,
                        )
                    nc.tensor.matmul(  # + b2 for every q (select sums to 1)
                        k[:],
                        ones[0:1, 0:128],
                        b2_sb[2][0:1, :],
                        start=False,
                        stop=True,
                    )
                    ksb = work.tile([128, NCD], fr, tag="ksb", name="ksb")
                    nc.vector.tensor_copy(ksb[:], k[:])
                    nc.tensor.matmul(
                        py[:],
                        oh_sb[:, qc * P : (qc + 1) * P],
                        ksb[:],
                        start=(qc == 0),
                        stop=(qc == QC2 - 1),
                    )
                _norm_and_store(nc, tc, work, AF, f32, py, out, 2, b)

    nc.compile()
    return nc


def _norm_and_store(nc, tc, work, AF, f32, py, out, l, b):
    sq = work.tile([128, NCD], f32, tag="sq", name="sq")
    ssq = work.tile([128, 1], f32, tag="ssq", name="ssq")
    nc.scalar.activation(sq[:], py[:], AF.Square, accum_out=ssq[:])
    nrm = work.tile([128, 1], f32, tag="nrm", name="nrm")
    nc.scalar.sqrt(nrm[:], ssq[:])
    nrm2 = work.tile([128, 1], f32, tag="nrm2", name="nrm2")
    nc.vector.tensor_scalar_add(nrm2[:], nrm[:], EPS)
    inv = work.tile([128, 1], f32, tag="inv", name="inv")
    nc.vector.reciprocal(inv[:], nrm2[:])
    yo = work.tile([128, NCD], f32, tag="yo", name="yo")
    nc.scalar.mul(yo[:], py[:], inv[:])
    nc.sync.dma_start(out[l, b], yo[:])


def _run(inputs, trace=False):
    from concourse.bass_utils import run_bass_kernel_spmd

    feats = [np.ascontiguousarray(np.asarray(inputs[f"feat{l}"])) for l in range(3)]
    idxs = [np.asarray(inputs[f"idx{l}"]).astype(np.int64) for l in range(3)]
    nc = _build(idxs)

    oh2 = np.zeros((8, 128, P), np.float32)
    for p, q in enumerate(idxs[2]):
        oh2[int(q) // 128, int(q) % 128, p] = 1.0

    in_maps = []
    for c in range(N_CORES):
        m = {"oh2": oh2}
        for l in range(3):
            m[f"feat{l}"] = feats[l][c * B_LOC : (c + 1) * B_LOC]
            m[f"w1_{l}"] = np.asarray(inputs[f"w1_{l}"])
            m[f"b1_{l}"] = np.asarray(inputs[f"b1_{l}"])
            m[f"w2_{l}"] = np.asarray(inputs[f"w2_{l}"])
            m[f"b2_{l}"] = np.asarray(inputs[f"b2_{l}"])
        in_maps.append(m)

    res = run_bass_kernel_spmd(
        nc, in_maps, core_ids=list(range(N_CORES)), trace=trace
    )
    full = np.concatenate([r["out"] for r in res.results], axis=1)
    return full.astype(np.float32), res


def kernel(**inputs) -> np.ndarray:
    out, _ = _run(inputs, trace=False)
    return out


# revision 20
# speedup vs baseline: 1.6181x; 1.0083x over previous
"""NPMLPHead sampling kernel for Trainium2 (Bass/Tile), SPMD over 8 cores.

Strategy (data-parallel over batch, per sharding hint):
  - B=16 images -> 2 per core; full (tiny) MLP weights replicated per core.
  - Levels 0/1 (sparse: 128 of 16K/4K positions): gather the patch vectors
    straight from HBM with strided DMAs (one DMA per patch; elements land
    transposed as xT[c, p] in SBUF). L0 rides HWDGE; L1 mostly SWDGE whose
    descriptors coalesce 16x, with a tail on the second HWDGE ring to
    balance the Pool sequencer against the DMA engines.
  - Level 2 (dense-ish: 128 of 1024): stream the whole shard and use
    associativity  y = S^T ((relu(W1^T T))^T W2)  -- layer-1 and layer-2
    computed over ALL positions keep the contraction dim on partitions in
    the native [C, HW] layout (no transposes), and the one-hot select S
    becomes a single PE matmul once q lands on partitions.
  - All matmuls in float32r (single-pass fp32, 4x faster; ~tf32 rounding).
  - L2-normalize rows with ACT/DVE ops, DMA out; host concatenates shards.
"""

import sys

sys.path.insert(0, "/opt/trn_rl_repo")

import numpy as np

B = 16
N_CORES = 8
B_LOC = B // N_CORES  # 2
P = 128  # NUM_PATCHES
NCD = 256  # MLP width
LEVELS = [(256, 128), (512, 64), (1024, 32)]  # (C, H) per level
EPS = 1e-7
L1_POOL_COUNT = 92  # L1 gather DMAs on SWDGE; the rest on HWDGE (balance)


def _build(idx_vals):
    """Build the per-core Bass kernel. idx_vals: 3 int arrays of 128 patch ids."""
    import concourse.bass as bass
    import concourse.tile as tile
    from concourse import bacc, mybir

    f32 = mybir.dt.float32
    fr = mybir.dt.float32r
    AF = mybir.ActivationFunctionType

    nc = bacc.Bacc(None)

    feats, w1s, b1s, w2s, b2s = [], [], [], [], []
    for l, (C, H) in enumerate(LEVELS):
        feats.append(
            nc.dram_tensor(f"feat{l}", [B_LOC, C, H, H], fr, kind="ExternalInput")
        )
        w1s.append(nc.dram_tensor(f"w1_{l}", [C, NCD], fr, kind="ExternalInput"))
        b1s.append(nc.dram_tensor(f"b1_{l}", [NCD], fr, kind="ExternalInput"))
        w2s.append(nc.dram_tensor(f"w2_{l}", [NCD, NCD], fr, kind="ExternalInput"))
        b2s.append(nc.dram_tensor(f"b2_{l}", [NCD], fr, kind="ExternalInput"))
    # one-hot select for level 2: oh[qc, ql, p] = (idx2[p] == qc*128+ql)
    oh = nc.dram_tensor("oh2", [8, 128, P], fr, kind="ExternalInput")
    out = nc.dram_tensor("out", [3, B_LOC, P, NCD], f32, kind="ExternalOutput")

    C2, H2 = LEVELS[2]
    HW2 = H2 * H2  # 1024
    NCH2 = C2 // 128  # 8
    QC2 = HW2 // 128  # 8 q-chunks

    with tile.TileContext(nc) as tc:
        with (
            tc.tile_pool(name="consts", bufs=1) as consts,
            tc.tile_pool(name="xt", bufs=1) as xtp,
            tc.tile_pool(name="work", bufs=4) as work,
            tc.tile_pool(name="psum", bufs=2, space=bass.MemorySpace.PSUM) as psum,
            tc.tile_pool(name="psum1", bufs=1, space=bass.MemorySpace.PSUM) as psum1,
        ):
            ones_f = consts.tile([1, 512], f32, tag="ones_f")
            nc.vector.memset(ones_f[:], 1.0)
            ones = consts.tile([1, 512], fr, tag="ones")
            nc.scalar.copy(ones[:], ones_f[:])

            w1_sb, w2_sb, b1_sb, b2_sb, xts = [], [], [], [], []
            for l, (C, H) in enumerate(LEVELS):
                n_ch = C // 128
                t = consts.tile([128, n_ch * NCD], fr, tag=f"w1_{l}", name=f"w1sb{l}")
                nc.sync.dma_start(
                    t[:].rearrange("cp (ch n) -> cp ch n", n=NCD),
                    w1s[l][:].rearrange("(ch cp) n -> cp ch n", cp=128),
                )
                w1_sb.append(t)
                t = consts.tile([128, 2 * NCD], fr, tag=f"w2_{l}", name=f"w2sb{l}")
                nc.scalar.dma_start(
                    t[:].rearrange("cp (ch n) -> cp ch n", n=NCD),
                    w2s[l][:].rearrange("(ch cp) n -> cp ch n", cp=128),
                )
                w2_sb.append(t)
                t = consts.tile([1, NCD], fr, tag=f"b1_{l}", name=f"b1sb{l}")
                nc.sync.dma_start(t[:], b1s[l][:].rearrange("(o n) -> o n", o=1))
                b1_sb.append(t)
                t = consts.tile([1, NCD], fr, tag=f"b2_{l}", name=f"b2sb{l}")
                nc.scalar.dma_start(t[:], b2s[l][:].rearrange("(o n) -> o n", o=1))
                b2_sb.append(t)
                if l < 2:
                    # gathered patches, transposed: [c_part, (b, chunk) x patch]
                    xts.append(
                        xtp.tile(
                            [128, B_LOC * n_ch * P], fr, tag=f"xt_{l}", name=f"xt{l}"
                        )
                    )

            oh_sb = consts.tile([128, QC2 * P], fr, tag="oh2")
            nc.scalar.dma_start(
                oh_sb[:].rearrange("ql (qc p) -> ql qc p", p=P),
                oh[:].rearrange("qc ql p -> ql qc p"),
            )

            # --- L2: stream whole shard first (overlaps gather phase) ---
            t2s = []
            for b in range(B_LOC):
                t2 = xtp.tile([128, NCH2 * HW2], fr, tag=f"t2_{b}", name=f"t2_{b}")
                (nc.scalar if b == 0 else nc.sync).dma_start(
                    t2[:].rearrange("cp (cc hw) -> cp cc hw", hw=HW2),
                    feats[2][b].rearrange("(cc cp) h w -> cp cc (h w)", cp=128),
                )
                t2s.append(t2)

            # --- L0/L1 sparse gather: one DMA per (level, patch) ---
            with nc.allow_non_contiguous_dma("sparse patch gather"):
                for l in (0, 1):
                    C, H = LEVELS[l]
                    n_ch = C // 128
                    src3 = feats[l][:].rearrange(
                        "b (ch cp) h w -> cp (b ch) (h w)", cp=128
                    )
                    dst3 = xts[l][:].rearrange("c (bc pp) -> c bc pp", pp=P)
                    for i, q in enumerate(idx_vals[l]):
                        if l == 0:
                            # L0: alternate the two HWDGE rings
                            eng = nc.sync if i % 2 == 0 else nc.scalar
                        else:
                            # L1: mostly SWDGE; tail split across HWDGE rings
                            if i < L1_POOL_COUNT:
                                eng = nc.gpsimd
                            else:
                                eng = nc.sync if i % 2 == 0 else nc.scalar
                        eng.dma_start(dst3[:, :, i], src3[:, :, int(q)])

            # --- MLP for L0/L1 (both images batched into N=256) ---
            for l in (0, 1):
                C, H = LEVELS[l]
                n_ch = C // 128
                x4 = xts[l][:].rearrange("c (b ch p) -> c ch b p", b=B_LOC, p=P)
                hts = []
                for half in range(2):
                    ph = psum.tile([128, B_LOC * P], f32, tag="ph", name="ph")
                    for ch in range(n_ch):
                        o = ch * NCD + half * 128
                        nc.tensor.matmul(
                            ph[:],
                            w1_sb[l][:, o : o + 128],
                            x4[:, ch],
                            start=(ch == 0),
                            stop=False,
                        )
                    nc.tensor.matmul(  # + b1 (rank-1)
                        ph[:],
                        b1_sb[l][0:1, half * 128 : half * 128 + 128],
                        ones[0:1, 0 : B_LOC * P],
                        start=False,
                        stop=True,
                    )
                    ht = work.tile([128, B_LOC * P], fr, tag="ht", name="ht")
                    nc.scalar.activation(ht[:], ph[:], AF.Relu)
                    hts.append(ht)

                for b in range(B_LOC):
                    py = psum.tile([128, NCD], f32, tag="py", name="py")
                    for half in range(2):
                        nc.tensor.matmul(
                            py[:],
                            hts[half][:, b * P : (b + 1) * P],
                            w2_sb[l][:, half * NCD : (half + 1) * NCD],
                            start=(half == 0),
                            stop=False,
                        )
                    nc.tensor.matmul(  # + b2 (rank-1)
                        py[:],
                        ones[0:1, 0:P],
                        b2_sb[l][0:1, :],
                        start=False,
                        stop=True,
                    )
                    _norm_and_store(nc, tc, work, AF, f32, py, out, l, b)

            # --- L2 full-compute: G=W1^T T (all q), H=relu(G), K=H^T W2,
            #     y = S^T K (one-hot select once q is on partitions) ---
            for b in range(B_LOC):
                t2 = t2s[b]
                h2 = xtp.tile([128, 2 * HW2], fr, tag=f"h2_{b}", name=f"h2_{b}")
                for half in range(2):
                    # two 512-wide q-slabs accumulate in parallel PSUM banks so
                    # each w1 stationary chunk is loaded once, not per slab
                    gs = [
                        psum1.tile([128, 512], f32, tag=f"g{qn}", name=f"g{qn}")
                        for qn in range(2)
                    ]
                    for cc in range(NCH2):
                        for qn in range(2):
                            nc.tensor.matmul(
                                gs[qn][:],
                                w1_sb[2][:, cc * NCD + half * 128 : cc * NCD + half * 128 + 128],
                                t2[:, cc * HW2 + qn * 512 : cc * HW2 + qn * 512 + 512],
                                start=(cc == 0),
                                stop=False,
                            )
                    for qn in range(2):
                        nc.tensor.matmul(  # + b1 broadcast over all q
                            gs[qn][:],
                            b1_sb[2][0:1, half * 128 : half * 128 + 128],
                            ones[0:1, 0:512],
                            start=False,
                            stop=True,
                        )
                        nc.scalar.activation(
                            h2[:, (half * 2 + qn) * 512 : (half * 2 + qn) * 512 + 512],
                            gs[qn][:],
                            AF.Relu,
                        )

                py = psum.tile([128, NCD], f32, tag="py", name="py2")  # shares "py" bank slots
                for qc in range(QC2):
                    k = psum.tile([128, NCD], f32, tag="k", name="k")
                    for half in range(2):
                        o = (half * 2 + qc // 4) * 512 + (qc % 4) * 128
                        nc.tensor.matmul(
                            k[:],
                            h2[:, o : o + 128],
                            w2_sb[2][:, half * NCD : (half + 1) * NCD],
                            start=(half == 0),
                            stop=False,
                        )
                    nc.tensor.matmul(  # + b2 for every q (select sums to 1)
                        k[:],
                        ones[0:1, 0:128],
                        b2_sb[2][0:1, :],
                        start=False,
                        stop=True,
                    )
                    ksb = work.tile([128, NCD], fr, tag="ksb", name="ksb")
                    nc.vector.tensor_copy(ksb[:], k[:])
                    nc.tensor.matmul(
                        py[:],
                        oh_sb[:, qc * P : (qc + 1) * P],
                        ksb[:],
                        start=(qc == 0),
                        stop=(qc == QC2 - 1),
                    )
                _norm_and_store(nc, tc, work, AF, f32, py, out, 2, b)

    nc.compile()
    return nc


def _norm_and_store(nc, tc, work, AF, f32, py, out, l, b):
    sq = work.tile([128, NCD], f32, tag="sq", name="sq")
    ssq = work.tile([128, 1], f32, tag="ssq", name="ssq")
    nc.scalar.activation(sq[:], py[:], AF.Square, accum_out=ssq[:])
    nrm = work.tile([128, 1], f32, tag="nrm", name="nrm")
    nc.scalar.sqrt(nrm[:], ssq[:])
    nrm2 = work.tile([128, 1], f32, tag="nrm2", name="nrm2")
    nc.vector.tensor_scalar_add(nrm2[:], nrm[:], EPS)
    inv = work.tile([128, 1], f32, tag="inv", name="inv")
    nc.vector.reciprocal(inv[:], nrm2[:])
    yo = work.tile([128, NCD], f32, tag="yo", name="yo")
    nc.scalar.mul(yo[:], py[:], inv[:])
    nc.sync.dma_start(out[l, b], yo[:])


def _run(inputs, trace=False):
    from concourse.bass_utils import run_bass_kernel_spmd

    feats = [
        np.ascontiguousarray(np.asarray(inputs[f"feat{l}"], dtype=np.float32))
        for l in range(3)
    ]
    idxs = [np.asarray(inputs[f"idx{l}"]).astype(np.int64) for l in range(3)]
    nc = _build(idxs)

    oh2 = np.zeros((8, 128, P), np.float32)
    for p, q in enumerate(idxs[2]):
        oh2[int(q) // 128, int(q) % 128, p] = 1.0

    in_maps = []
    for c in range(N_CORES):
        m = {"oh2": oh2}
        for l in range(3):
            m[f"feat{l}"] = feats[l][c * B_LOC : (c + 1) * B_LOC]
            m[f"w1_{l}"] = np.asarray(inputs[f"w1_{l}"], dtype=np.float32)
            m[f"b1_{l}"] = np.asarray(inputs[f"b1_{l}"], dtype=np.float32)
            m[f"w2_{l}"] = np.asarray(inputs[f"w2_{l}"], dtype=np.float32)
            m[f"b2_{l}"] = np.asarray(inputs[f"b2_{l}"], dtype=np.float32)
        in_maps.append(m)

    res = run_bass_kernel_spmd(
        nc, in_maps, core_ids=list(range(N_CORES)), trace=trace
    )
    full = np.concatenate([r["out"] for r in res.results], axis=1)
    return full.astype(np.float32), res


def kernel(**inputs) -> np.ndarray:
    out, _ = _run(inputs, trace=False)
    return out
